# revision 35
# baseline (speedup 1.0000x reference)
"""Multi-head causal attention (B=8, T=1024, C=1024, H=16, hs=64) on 8 trn2 cores.

Data-parallel over batch: core b computes full attention for x[b].

Device algorithm (per core), all matmuls bf16 inputs / fp32 PSUM accum:
  - xT [C, T] resident in SBUF (host pre-transposed, bf16).
  - v computed in two 4-pair groups interleaved with the pair loop:
    v_all[s, head, s_tile, 0:64] plus a ones column at index 64 so the AV
    matmul also emits softmax denominators (normalization happens on HOST).
  - per head-pair (2 heads packed on partitions): qT, kT = W^T @ xT -> [128, T].
  - scores transposed: scT[s_tile, (head, t)] = kT_chunk^T @ qT for causal
    spans only; BOTH heads land in one [128, 2, 512] f32 PSUM tile so a
    single ScalarE exp (scale=1/8) covers both heads per span (fewer,
    bigger ACT calls - ScalarE is the co-bottleneck).
  - causal masking of the diagonal 128x128 block is ONE in-place GpSimd
    affine_select per (pair, s_tile) on the bf16 exp tile (keeps DVE free).
  - out^T[65, t] accumulated over s chunks per (head, t-half): lhsT = [v | 1],
    rhs = exp^T slice. Row 64 = sum(exp) = softmax denominator.
  - [65, 512] result copied PSUM->SBUF on DVE and DMAed to out[h, :, half].
    Host divides rows 0:64 by row 64 and transposes - no on-device
    normalization (kills the reciprocal/broadcast/divide chain + tail).
Software pipelining: AV(p) is emitted one pair late (at pair p+1) so the
PE never waits on ScalarE exp; V j-half groups are spread at pairs
0/1/3/5, and the last pair's score spans are staggered between AV(6)'s
halves so exp(7) overlaps AV work instead of trailing it.
"""

import numpy as np
import ml_dtypes

import concourse.bass as bass
import concourse.mybir as mybir
from concourse import bacc
from concourse.tile import TileContext
from concourse.bass import ds, ts
from concourse.bass_utils import run_bass_kernel_spmd

BF16 = mybir.dt.bfloat16
F32 = mybir.dt.float32

B, T, C, H, HS = 8, 1024, 1024, 16, 64
P = 128
CK = C // P       # 8 contraction chunks
TT = T // P       # 8 t tiles
PAIRS = H // 2    # 8 head pairs
HALF = 512

_BUILT = None


def build_nc():
    nc = bacc.Bacc("TRN2", target_bir_lowering=False, debug=False)
    # [p, c, t] : xT[C, T] chunked; partition p, chunk c -> row 128c+p of xT
    xt = nc.dram_tensor("xt", [P, CK, T], BF16, kind="ExternalInput")
    # [proj(q,k), pair, p, c, f] : lhsT chunks, f = 2 heads x 64 stacked
    wqk = nc.dram_tensor("wqk", [2, PAIRS, P, CK, P], BF16, kind="ExternalInput")
    # [p, c, group(2), 8 heads x 64]
    wv = nc.dram_tensor("wv", [P, CK, 2, 4 * P], BF16, kind="ExternalInput")
    # out^T per head: [head, 64 d + 1 denom, t]; host divides + transposes
    out = nc.dram_tensor("out", [H, HS + 1, T], F32, kind="ExternalOutput")

    with TileContext(nc) as tc:
        with (
            tc.tile_pool(name="const", bufs=1) as constp,
            tc.tile_pool(name="wpool", bufs=6) as wpool,
            tc.tile_pool(name="qkpool", bufs=6) as qkp,
            tc.tile_pool(name="exppool", bufs=16) as expp,
            tc.tile_pool(name="smallpool", bufs=6) as smallp,
            tc.tile_pool(name="psA", bufs=2, space="PSUM") as psA,
            tc.tile_pool(name="psSc", bufs=2, space="PSUM") as psSc,
            tc.tile_pool(name="psV", bufs=2, space="PSUM") as psV,
        ):
            wtiles = {}

            def prefetch_w(pair, eng=None):
                if pair >= PAIRS:
                    return
                e = eng or nc.sync
                wq_sb = wpool.tile([P, CK, P], BF16, tag="w", name=f"wq{pair}")
                e.dma_start(wq_sb[:, :, :], wqk[0, pair, :, :, :])
                wk_sb = wpool.tile([P, CK, P], BF16, tag="w", name=f"wk{pair}")
                e.dma_start(wk_sb[:, :, :], wqk[1, pair, :, :, :])
                wtiles[pair] = (wq_sb, wk_sb)

            prefetch_w(0)
            xt_sb = constp.tile([P, CK, T], BF16)
            # qk(0) g=0 needs the first halves of ALL chunks; issue those
            # first as separate DMAs so the first matmuls start sooner
            for c in range(CK):
                nc.sync.dma_start(xt_sb[:, c, 0:HALF], xt[:, c, 0:HALF])
            for c in range(CK):
                nc.sync.dma_start(xt_sb[:, c, HALF:T], xt[:, c, HALF:T])
            prefetch_w(1)
            wv_sb = constp.tile([P, CK, 2, 4 * P], BF16)
            for c in range(CK):
                nc.sync.dma_start(wv_sb[:, c, :, :], wv[:, c, :, :])
            prefetch_w(2)
            # [s_p, head, s_tile, 64 v cols + 1 ones col]
            v_all = constp.tile([P, H, TT, HS + 1], BF16)
            nc.gpsimd.memset(v_all[:, :, :, HS:HS + 1], 1.0)

            qk = {}

            def emit_qk(pair, g_outer=False):
                wq_sb, wk_sb = wtiles.pop(pair)
                qT = qkp.tile([P, T], BF16, tag="qk", name=f"q{pair}")
                kT = qkp.tile([P, T], BF16, tag="qk", name=f"k{pair}")
                # g_outer (pair 0): do both projections' g=0 halves first so
                # the start-of-kernel matmuls only need the first xt halves
                order = (((wq_sb, qT, 0), (wk_sb, kT, 0),
                          (wq_sb, qT, 1), (wk_sb, kT, 1)) if g_outer else
                         ((wq_sb, qT, 0), (wq_sb, qT, 1),
                          (wk_sb, kT, 0), (wk_sb, kT, 1)))
                for wsb, dst, g in order:
                    pp = psA.tile([P, HALF], F32, tag="ps",
                                  name=f"pp{pair}_{id(dst)}_{g}")
                    for c in range(CK):
                        nc.tensor.matmul(
                            pp[:, :],
                            wsb[:, c, :],
                            xt_sb[:, c, ds(HALF * g, HALF)],
                            start=(c == 0),
                            stop=(c == CK - 1),
                        )
                    nc.vector.tensor_copy(dst[:, ds(HALF * g, HALF)], pp[:, :])
                qk[pair] = (qT, kT)

            def emit_v_group(g, jr):
                # v for heads 8g..8g+7 (pairs 4g..4g+3), t tiles in jr
                for j in jr:
                    pv = psA.tile([P, HALF], F32, tag="ps", name=f"pv{g}_{j}")
                    for c in range(CK):
                        nc.tensor.matmul(
                            pv[:, :],
                            xt_sb[:, c, ts(j, P)],
                            wv_sb[:, c, g, :],
                            start=(c == 0),
                            stop=(c == CK - 1),
                        )
                    # pv cols are (8 heads of group) x 64 in order
                    nc.vector.tensor_copy(
                        v_all[:, ds(8 * g, 8), j, 0:HS],
                        pv.rearrange("p (h d) -> p h d", d=HS),
                    )

            es_tiles = {}

            def emit_scores_exp(pair, ir=range(TT), part="ab"):
                if pair not in es_tiles:
                    qT, kT = qk[pair]
                    es = [expp.tile([P, 2, T], BF16, tag="exp",
                                    name=f"e{pair}_{i}") for i in range(TT)]
                    es_tiles[pair] = (qT, kT, es)
                qT, kT, es = es_tiles[pair]
                for i in ir:
                    t0 = P * i
                    spans = [(t0, HALF), (HALF, T)] if t0 < HALF else [(t0, T)]
                    if t0 < HALF and part == "a":
                        spans = spans[:1]
                    elif t0 < HALF and part == "b":
                        spans = spans[1:]
                    for a, b in spans:
                        sc = psSc.tile([P, 2, HALF], F32, tag="sc",
                                       name=f"sc{pair}_{i}_{a}")
                        for w in range(2):
                            nc.tensor.matmul(
                                sc[:, w, 0:b - a],
                                kT[ds(HS * w, HS), ds(t0, P)],
                                qT[ds(HS * w, HS), ds(a, b - a)],
                            )
                        # one exp for BOTH heads (scale = 1/sqrt(hs))
                        nc.scalar.activation(
                            es[i][:, :, a:b],
                            sc[:, :, 0:b - a],
                            mybir.ActivationFunctionType.Exp,
                            scale=HS ** -0.5,
                        )
                        if a == t0:
                            # in-place causal mask of the diagonal block:
                            # keep where t - s >= 0, else 0
                            nc.gpsimd.affine_select(
                                out=es[i][:, :, t0:t0 + P],
                                in_=es[i][:, :, t0:t0 + P],
                                pattern=[[0, 2], [1, P]],
                                compare_op=mybir.AluOpType.is_ge,
                                fill=0.0,
                                base=0,
                                channel_multiplier=-1,
                            )

            def emit_av(pair, hhs=(0, 1), split_copies=False, dma_eng=None):
                de = dma_eng or nc.sync
                _, _, es = es_tiles[pair]
                for hh in hhs:
                    avp = [psV.tile([HS + 1, HALF], F32, tag="av",
                                    name=f"av{pair}_{hh}_{w}") for w in range(2)]
                    contrib = [i for i in range(TT) if P * i < HALF * (hh + 1)]
                    for idx, i in enumerate(contrib):
                        g0 = max(HALF * hh, P * i)
                        g1 = HALF * (hh + 1)
                        for w in range(2):
                            nc.tensor.matmul(
                                avp[w][:, ds(g0 - HALF * hh, g1 - g0)],
                                v_all[:, 2 * pair + w, i, :],
                                es[i][:, w, ds(g0, g1 - g0)],
                                start=(idx == 0),
                                stop=(idx == len(contrib) - 1),
                            )
                    for w in range(2):
                        h = 2 * pair + w
                        avs = smallp.tile([HS + 1, HALF], F32, tag="avs",
                                          name=f"avs{pair}_{hh}_{w}")
                        if split_copies and hh == 1:
                            # half-granular copy->DMA so the last out DMA
                            # starts before the full PSUM drain finishes
                            for q in range(2):
                                nc.vector.tensor_copy(
                                    avs[:, ds(256 * q, 256)],
                                    avp[w][:, ds(256 * q, 256)])
                                de.dma_start(
                                    out[h, :, ds(HALF * hh + 256 * q, 256)],
                                    avs[:, ds(256 * q, 256)])
                        else:
                            nc.vector.tensor_copy(avs[:, :], avp[w][:, :])
                            de.dma_start(out[h, :, ds(HALF * hh, HALF)],
                                         avs[:, :])

            # schedule: AV(p) emitted one pair late so PE never waits on
            # ScalarE exp; V-proj groups split in j-halves spread at pairs
            # 0/1/3/5 (vg0 complete before AV(0) at pair 1; vg1 before
            # AV(4) at pair 5).
            V_AT = {0: (0, range(0, 4)), 1: (0, range(4, 8)),
                    3: (1, range(0, 4)), 5: (1, range(4, 8))}
            emit_qk(0)
            for pair in range(PAIRS - 1):
                emit_scores_exp(pair)
                prefetch_w(pair + 3)
                emit_qk(pair + 1)
                if pair in V_AT:
                    emit_v_group(*V_AT[pair])
                if pair >= 1:
                    emit_av(pair - 1)
            # pair 7 tail: stagger the last pair's score spans between
            # AV(6)'s halves so exp(7) overlaps AV work instead of
            # trailing it
            # the last two AV groups' out-DMA triggers go via ScalarE: by
            # then the exp stream is done, ScalarE is idle, and the Sync
            # queue's ~0.7us/trigger serialization is off the critical path
            emit_scores_exp(7, range(0, 4))
            emit_av(6, (0,))
            emit_scores_exp(7, range(4, 8))
            emit_av(6, (1,), dma_eng=nc.scalar)
            emit_av(7, split_copies=True, dma_eng=nc.scalar)
    nc.compile()
    return nc


def get_nc():
    global _BUILT
    if _BUILT is None:
        _BUILT = build_nc()
    return _BUILT


def prep_inputs(x, Wq, Wk, Wv):
    """Host-side shard + layout prep. Returns in_maps (one dict per core)."""
    x = np.asarray(x, dtype=np.float32)
    Wq = np.asarray(Wq, dtype=np.float32)
    Wk = np.asarray(Wk, dtype=np.float32)
    Wv = np.asarray(Wv, dtype=np.float32)
    bf = ml_dtypes.bfloat16

    # xT[b]: [C, T] -> [p, c, t] with row 128c+p
    xts = []
    for b in range(B):
        xT = np.ascontiguousarray(x[b].T)          # [C, T]
        xts.append(xT.reshape(CK, P, T).transpose(1, 0, 2).astype(bf))

    def pack_pairs(W):
        # [H, C, hs] -> [pair, C, 128] -> [pair, p, c, f]
        Wp = W.reshape(PAIRS, 2, C, HS).transpose(0, 2, 1, 3).reshape(PAIRS, C, P)
        return Wp.reshape(PAIRS, CK, P, P).transpose(0, 2, 1, 3)  # [pair, p, c, f]

    wq_p = pack_pairs(Wq)
    wk_p = pack_pairs(Wk)
    wqk_host = np.stack([wq_p, wk_p], axis=0).astype(bf)  # [2, pair, p, c, f]
    # wv: [p, c, group(2), 512] with cols = 8 heads x 64
    wv_g = Wv.reshape(2, 8, C, HS).transpose(2, 0, 1, 3).reshape(C, 2, 4 * P)
    wv_host = np.ascontiguousarray(
        wv_g.reshape(CK, P, 2, 4 * P).transpose(1, 0, 2, 3)).astype(bf)

    return [
        {"xt": np.ascontiguousarray(xts[b]), "wqk": wqk_host, "wv": wv_host}
        for b in range(B)
    ]


def run_on_device(in_maps, **kwargs):
    nc = get_nc()
    return run_bass_kernel_spmd(nc, in_maps, list(range(B)), **kwargs)


def assemble(core_out):
    """[H, 65, T] raw out^T -> normalize on host -> [T, H*HS]."""
    o = np.asarray(core_out, dtype=np.float32)
    num = o[:, :HS, :]                       # [H, 64, T]
    den = o[:, HS:HS + 1, :]                 # [H, 1, T]
    y = num / den
    return np.ascontiguousarray(y.transpose(2, 0, 1).reshape(T, H * HS))


def kernel(x, Wq, Wk, Wv):
    in_maps = prep_inputs(x, Wq, Wk, Wv)
    res = run_on_device(in_maps)
    return np.stack([assemble(res.results[b]["out"]) for b in range(B)], axis=0)


# revision 36
# speedup vs baseline: 1.0124x; 1.0124x over previous
"""Multi-head causal attention (B=8, T=1024, C=1024, H=16, hs=64) on 8 trn2 cores.

Data-parallel over batch: core b computes full attention for x[b].

Device algorithm (per core), all matmuls bf16 inputs / fp32 PSUM accum:
  - xT [C, T] resident in SBUF (host pre-transposed, bf16).
  - v computed in two 4-pair groups interleaved with the pair loop:
    v_all[s, head, s_tile, 0:64] plus a ones column at index 64 so the AV
    matmul also emits softmax denominators (normalization happens on HOST).
  - per head-pair (2 heads packed on partitions): qT, kT = W^T @ xT -> [128, T].
  - scores transposed: scT[s_tile, (head, t)] = kT_chunk^T @ qT for causal
    spans only; BOTH heads land in one [128, 2, 512] f32 PSUM tile so a
    single ScalarE exp (scale=1/8) covers both heads per span (fewer,
    bigger ACT calls - ScalarE is the co-bottleneck).
  - causal masking of the diagonal 128x128 block is ONE in-place GpSimd
    affine_select per (pair, s_tile) on the bf16 exp tile (keeps DVE free).
  - out^T[65, t] accumulated over s chunks per (head, t-half): lhsT = [v | 1],
    rhs = exp^T slice. Row 64 = sum(exp) = softmax denominator.
  - [65, 512] result copied PSUM->SBUF on DVE and DMAed to out[h, :, half].
    Host divides rows 0:64 by row 64 and transposes - no on-device
    normalization (kills the reciprocal/broadcast/divide chain + tail).
Software pipelining: AV(p) is emitted one pair late (at pair p+1) so the
PE never waits on ScalarE exp; V j-half groups are spread at pairs
0/1/3/5, and the last pair's score spans are staggered between AV(6)'s
halves so exp(7) overlaps AV work instead of trailing it.
"""

import numpy as np
import ml_dtypes

import concourse.bass as bass
import concourse.mybir as mybir
from concourse import bacc
from concourse.tile import TileContext
from concourse.bass import ds, ts
from concourse.bass_utils import run_bass_kernel_spmd

BF16 = mybir.dt.bfloat16
F32 = mybir.dt.float32

B, T, C, H, HS = 8, 1024, 1024, 16, 64
P = 128
CK = C // P       # 8 contraction chunks
TT = T // P       # 8 t tiles
PAIRS = H // 2    # 8 head pairs
HALF = 512

_BUILT = None


def build_nc():
    nc = bacc.Bacc("TRN2", target_bir_lowering=False, debug=False)
    # [p, c, t] : xT[C, T] chunked; partition p, chunk c -> row 128c+p of xT
    xt = nc.dram_tensor("xt", [P, CK, T], BF16, kind="ExternalInput")
    # [proj(q,k), pair, p, c, f] : lhsT chunks, f = 2 heads x 64 stacked
    wqk = nc.dram_tensor("wqk", [2, PAIRS, P, CK, P], BF16, kind="ExternalInput")
    # [p, c, group(2), 8 heads x 64]
    wv = nc.dram_tensor("wv", [P, CK, 2, 4 * P], BF16, kind="ExternalInput")
    # out^T per head: [head, 64 d + 1 denom, t]; host divides + transposes
    out = nc.dram_tensor("out", [H, HS + 1, T], F32, kind="ExternalOutput")

    with TileContext(nc) as tc:
        with (
            tc.tile_pool(name="const", bufs=1) as constp,
            tc.tile_pool(name="wpool", bufs=6) as wpool,
            tc.tile_pool(name="qkpool", bufs=6) as qkp,
            tc.tile_pool(name="exppool", bufs=16) as expp,
            tc.tile_pool(name="smallpool", bufs=6) as smallp,
            tc.tile_pool(name="psA", bufs=2, space="PSUM") as psA,
            tc.tile_pool(name="psSc", bufs=2, space="PSUM") as psSc,
            tc.tile_pool(name="psV", bufs=2, space="PSUM") as psV,
        ):
            wtiles = {}

            def prefetch_w(pair, eng=None):
                if pair >= PAIRS:
                    return
                e = eng or nc.sync
                wq_sb = wpool.tile([P, CK, P], BF16, tag="w", name=f"wq{pair}")
                e.dma_start(wq_sb[:, :, :], wqk[0, pair, :, :, :])
                wk_sb = wpool.tile([P, CK, P], BF16, tag="w", name=f"wk{pair}")
                e.dma_start(wk_sb[:, :, :], wqk[1, pair, :, :, :])
                wtiles[pair] = (wq_sb, wk_sb)

            prefetch_w(0)
            xt_sb = constp.tile([P, CK, T], BF16)
            # qk(0) g=0 needs the first halves of ALL chunks; issue those
            # first as separate DMAs so the first matmuls start sooner
            for c in range(CK):
                nc.sync.dma_start(xt_sb[:, c, 0:HALF], xt[:, c, 0:HALF])
            for c in range(CK):
                nc.sync.dma_start(xt_sb[:, c, HALF:T], xt[:, c, HALF:T])
            prefetch_w(1)
            wv_sb = constp.tile([P, CK, 2, 4 * P], BF16)
            for c in range(CK):
                nc.sync.dma_start(wv_sb[:, c, :, :], wv[:, c, :, :])
            prefetch_w(2)
            # [s_p, head, s_tile, 64 v cols + 1 ones col]
            v_all = constp.tile([P, H, TT, HS + 1], BF16)
            nc.gpsimd.memset(v_all[:, :, :, HS:HS + 1], 1.0)

            qk = {}

            def emit_qk(pair, g_outer=False):
                wq_sb, wk_sb = wtiles.pop(pair)
                qT = qkp.tile([P, T], BF16, tag="qk", name=f"q{pair}")
                kT = qkp.tile([P, T], BF16, tag="qk", name=f"k{pair}")
                # g_outer (pair 0): do both projections' g=0 halves first so
                # the start-of-kernel matmuls only need the first xt halves
                order = (((wq_sb, qT, 0), (wk_sb, kT, 0),
                          (wq_sb, qT, 1), (wk_sb, kT, 1)) if g_outer else
                         ((wq_sb, qT, 0), (wq_sb, qT, 1),
                          (wk_sb, kT, 0), (wk_sb, kT, 1)))
                for wsb, dst, g in order:
                    pp = psA.tile([P, HALF], F32, tag="ps",
                                  name=f"pp{pair}_{id(dst)}_{g}")
                    for c in range(CK):
                        nc.tensor.matmul(
                            pp[:, :],
                            wsb[:, c, :],
                            xt_sb[:, c, ds(HALF * g, HALF)],
                            start=(c == 0),
                            stop=(c == CK - 1),
                        )
                    nc.vector.tensor_copy(dst[:, ds(HALF * g, HALF)], pp[:, :])
                qk[pair] = (qT, kT)

            def emit_v_group(g, jr):
                # v for heads 8g..8g+7 (pairs 4g..4g+3), t tiles in jr
                for j in jr:
                    pv = psA.tile([P, HALF], F32, tag="ps", name=f"pv{g}_{j}")
                    for c in range(CK):
                        nc.tensor.matmul(
                            pv[:, :],
                            xt_sb[:, c, ts(j, P)],
                            wv_sb[:, c, g, :],
                            start=(c == 0),
                            stop=(c == CK - 1),
                        )
                    # pv cols are (8 heads of group) x 64 in order
                    nc.vector.tensor_copy(
                        v_all[:, ds(8 * g, 8), j, 0:HS],
                        pv.rearrange("p (h d) -> p h d", d=HS),
                    )

            es_tiles = {}

            def emit_scores_exp(pair, ir=range(TT), part="ab"):
                if pair not in es_tiles:
                    qT, kT = qk[pair]
                    es = [expp.tile([P, 2, T], BF16, tag="exp",
                                    name=f"e{pair}_{i}") for i in range(TT)]
                    es_tiles[pair] = (qT, kT, es)
                qT, kT, es = es_tiles[pair]
                for i in ir:
                    t0 = P * i
                    spans = [(t0, HALF), (HALF, T)] if t0 < HALF else [(t0, T)]
                    if t0 < HALF and part == "a":
                        spans = spans[:1]
                    elif t0 < HALF and part == "b":
                        spans = spans[1:]
                    for a, b in spans:
                        sc = psSc.tile([P, 2, HALF], F32, tag="sc",
                                       name=f"sc{pair}_{i}_{a}")
                        for w in range(2):
                            nc.tensor.matmul(
                                sc[:, w, 0:b - a],
                                kT[ds(HS * w, HS), ds(t0, P)],
                                qT[ds(HS * w, HS), ds(a, b - a)],
                            )
                        # one exp for BOTH heads (scale = 1/sqrt(hs))
                        nc.scalar.activation(
                            es[i][:, :, a:b],
                            sc[:, :, 0:b - a],
                            mybir.ActivationFunctionType.Exp,
                            scale=HS ** -0.5,
                        )
                        if a == t0:
                            # in-place causal mask of the diagonal block:
                            # keep where t - s >= 0, else 0
                            nc.gpsimd.affine_select(
                                out=es[i][:, :, t0:t0 + P],
                                in_=es[i][:, :, t0:t0 + P],
                                pattern=[[0, 2], [1, P]],
                                compare_op=mybir.AluOpType.is_ge,
                                fill=0.0,
                                base=0,
                                channel_multiplier=-1,
                            )

            def emit_av(pair, hhs=(0, 1), split_copies=False):
                _, _, es = es_tiles[pair]
                for hh in hhs:
                    avp = [psV.tile([HS + 1, HALF], F32, tag="av",
                                    name=f"av{pair}_{hh}_{w}") for w in range(2)]
                    contrib = [i for i in range(TT) if P * i < HALF * (hh + 1)]
                    for idx, i in enumerate(contrib):
                        g0 = max(HALF * hh, P * i)
                        g1 = HALF * (hh + 1)
                        for w in range(2):
                            nc.tensor.matmul(
                                avp[w][:, ds(g0 - HALF * hh, g1 - g0)],
                                v_all[:, 2 * pair + w, i, :],
                                es[i][:, w, ds(g0, g1 - g0)],
                                start=(idx == 0),
                                stop=(idx == len(contrib) - 1),
                            )
                    for w in range(2):
                        h = 2 * pair + w
                        avs = smallp.tile([HS + 1, HALF], F32, tag="avs",
                                          name=f"avs{pair}_{hh}_{w}")
                        if split_copies and hh == 1:
                            # half-granular copy->DMA so the last out DMA
                            # starts before the full PSUM drain finishes
                            for q in range(2):
                                nc.vector.tensor_copy(
                                    avs[:, ds(256 * q, 256)],
                                    avp[w][:, ds(256 * q, 256)])
                                nc.sync.dma_start(
                                    out[h, :, ds(HALF * hh + 256 * q, 256)],
                                    avs[:, ds(256 * q, 256)])
                        else:
                            nc.vector.tensor_copy(avs[:, :], avp[w][:, :])
                            nc.sync.dma_start(out[h, :, ds(HALF * hh, HALF)],
                                              avs[:, :])

            # schedule: AV(p) emitted one pair late so PE never waits on
            # ScalarE exp; V-proj groups split in j-halves spread at pairs
            # 0/1/3/5 (vg0 complete before AV(0) at pair 1; vg1 before
            # AV(4) at pair 5).
            V_AT = {0: (0, range(0, 4)), 1: (0, range(4, 8)),
                    3: (1, range(0, 4)), 5: (1, range(4, 8))}
            emit_qk(0)
            for pair in range(PAIRS - 1):
                emit_scores_exp(pair)
                prefetch_w(pair + 3)
                emit_qk(pair + 1)
                if pair in V_AT:
                    emit_v_group(*V_AT[pair])
                if pair >= 1:
                    emit_av(pair - 1)
            # pair 7 tail: stagger the last pair's score spans between
            # AV(6)'s halves so exp(7) overlaps AV work instead of
            # trailing it
            emit_scores_exp(7, range(0, 4))
            emit_av(6, (0,))
            emit_scores_exp(7, range(4, 8))
            emit_av(6, (1,))
            emit_av(7)
    nc.compile()
    return nc


def get_nc():
    global _BUILT
    if _BUILT is None:
        _BUILT = build_nc()
    return _BUILT


def prep_inputs(x, Wq, Wk, Wv):
    """Host-side shard + layout prep. Returns in_maps (one dict per core)."""
    x = np.asarray(x, dtype=np.float32)
    Wq = np.asarray(Wq, dtype=np.float32)
    Wk = np.asarray(Wk, dtype=np.float32)
    Wv = np.asarray(Wv, dtype=np.float32)
    bf = ml_dtypes.bfloat16

    # xT[b]: [C, T] -> [p, c, t] with row 128c+p
    xts = []
    for b in range(B):
        xT = np.ascontiguousarray(x[b].T)          # [C, T]
        xts.append(xT.reshape(CK, P, T).transpose(1, 0, 2).astype(bf))

    def pack_pairs(W):
        # [H, C, hs] -> [pair, C, 128] -> [pair, p, c, f]
        Wp = W.reshape(PAIRS, 2, C, HS).transpose(0, 2, 1, 3).reshape(PAIRS, C, P)
        return Wp.reshape(PAIRS, CK, P, P).transpose(0, 2, 1, 3)  # [pair, p, c, f]

    wq_p = pack_pairs(Wq)
    wk_p = pack_pairs(Wk)
    wqk_host = np.stack([wq_p, wk_p], axis=0).astype(bf)  # [2, pair, p, c, f]
    # wv: [p, c, group(2), 512] with cols = 8 heads x 64
    wv_g = Wv.reshape(2, 8, C, HS).transpose(2, 0, 1, 3).reshape(C, 2, 4 * P)
    wv_host = np.ascontiguousarray(
        wv_g.reshape(CK, P, 2, 4 * P).transpose(1, 0, 2, 3)).astype(bf)

    return [
        {"xt": np.ascontiguousarray(xts[b]), "wqk": wqk_host, "wv": wv_host}
        for b in range(B)
    ]


def run_on_device(in_maps, **kwargs):
    nc = get_nc()
    return run_bass_kernel_spmd(nc, in_maps, list(range(B)), **kwargs)


def assemble(core_out):
    """[H, 65, T] raw out^T -> normalize on host -> [T, H*HS]."""
    o = np.asarray(core_out, dtype=np.float32)
    num = o[:, :HS, :]                       # [H, 64, T]
    den = o[:, HS:HS + 1, :]                 # [H, 1, T]
    y = num / den
    return np.ascontiguousarray(y.transpose(2, 0, 1).reshape(T, H * HS))


def kernel(x, Wq, Wk, Wv):
    in_maps = prep_inputs(x, Wq, Wk, Wv)
    res = run_on_device(in_maps)
    return np.stack([assemble(res.results[b]["out"]) for b in range(B)], axis=0)


# revision 40
# speedup vs baseline: 1.0163x; 1.0038x over previous
"""Multi-head causal attention (B=8, T=1024, C=1024, H=16, hs=64) on 8 trn2 cores.

Data-parallel over batch: core b computes full attention for x[b].

Device algorithm (per core), all matmuls bf16 inputs / fp32 PSUM accum:
  - xT [C, T] resident in SBUF (host pre-transposed, bf16).
  - v computed in two 4-pair groups interleaved with the pair loop:
    v_all[s, head, s_tile, 0:64] plus a ones column at index 64 so the AV
    matmul also emits softmax denominators (normalization happens on HOST).
  - per head-pair (2 heads packed on partitions): qT, kT = W^T @ xT -> [128, T].
  - scores transposed: scT[s_tile, (head, t)] = kT_chunk^T @ qT for causal
    spans only; BOTH heads land in one [128, 2, 512] f32 PSUM tile so a
    single ScalarE exp (scale=1/8) covers both heads per span (fewer,
    bigger ACT calls - ScalarE is the co-bottleneck).
  - causal masking of the diagonal 128x128 block is ONE in-place GpSimd
    affine_select per (pair, s_tile) on the bf16 exp tile (keeps DVE free).
  - out^T[65, t] accumulated over s chunks per (head, t-half): lhsT = [v | 1],
    rhs = exp^T slice. Row 64 = sum(exp) = softmax denominator.
  - [65, 512] result copied PSUM->SBUF on DVE and DMAed to out[h, :, half].
    Host divides rows 0:64 by row 64 and transposes - no on-device
    normalization (kills the reciprocal/broadcast/divide chain + tail).
Software pipelining: AV(p) is emitted one pair late (at pair p+1) so the
PE never waits on ScalarE exp; V j-half groups are spread at pairs
0/1/3/5, and the last pair's score spans are staggered between AV(6)'s
halves so exp(7) overlaps AV work instead of trailing it.
"""

import numpy as np
import ml_dtypes

import concourse.bass as bass
import concourse.mybir as mybir
from concourse import bacc
from concourse.tile import TileContext
from concourse.bass import ds, ts
from concourse.bass_utils import run_bass_kernel_spmd

BF16 = mybir.dt.bfloat16
F32 = mybir.dt.float32

B, T, C, H, HS = 8, 1024, 1024, 16, 64
P = 128
CK = C // P       # 8 contraction chunks
TT = T // P       # 8 t tiles
PAIRS = H // 2    # 8 head pairs
HALF = 512

_BUILT = None


def build_nc():
    nc = bacc.Bacc("TRN2", target_bir_lowering=False, debug=False)
    # [p, c, t] : xT[C, T] chunked; partition p, chunk c -> row 128c+p of xT
    xt = nc.dram_tensor("xt", [P, CK, T], BF16, kind="ExternalInput")
    # [proj(q,k), pair, p, c, f] : lhsT chunks, f = 2 heads x 64 stacked
    wqk = nc.dram_tensor("wqk", [2, PAIRS, P, CK, P], BF16, kind="ExternalInput")
    # [p, c, group(2), 8 heads x 64]
    wv = nc.dram_tensor("wv", [P, CK, 2, 4 * P], BF16, kind="ExternalInput")
    # out^T per head: [head, 64 d + 1 denom, t]; host divides + transposes
    out = nc.dram_tensor("out", [H, HS + 1, T], BF16, kind="ExternalOutput")

    with TileContext(nc) as tc:
        with (
            tc.tile_pool(name="const", bufs=1) as constp,
            tc.tile_pool(name="wpool", bufs=6) as wpool,
            tc.tile_pool(name="qkpool", bufs=6) as qkp,
            tc.tile_pool(name="exppool", bufs=16) as expp,
            tc.tile_pool(name="smallpool", bufs=6) as smallp,
            tc.tile_pool(name="psA", bufs=2, space="PSUM") as psA,
            tc.tile_pool(name="psSc", bufs=2, space="PSUM") as psSc,
            tc.tile_pool(name="psV", bufs=2, space="PSUM") as psV,
        ):
            wtiles = {}

            def prefetch_w(pair, eng=None):
                if pair >= PAIRS:
                    return
                e = eng or nc.sync
                wq_sb = wpool.tile([P, CK, P], BF16, tag="w", name=f"wq{pair}")
                e.dma_start(wq_sb[:, :, :], wqk[0, pair, :, :, :])
                wk_sb = wpool.tile([P, CK, P], BF16, tag="w", name=f"wk{pair}")
                e.dma_start(wk_sb[:, :, :], wqk[1, pair, :, :, :])
                wtiles[pair] = (wq_sb, wk_sb)

            prefetch_w(0)
            xt_sb = constp.tile([P, CK, T], BF16)
            # qk(0) g=0 needs the first halves of ALL chunks; issue those
            # first as separate DMAs so the first matmuls start sooner
            for c in range(CK):
                nc.sync.dma_start(xt_sb[:, c, 0:HALF], xt[:, c, 0:HALF])
            for c in range(CK):
                nc.sync.dma_start(xt_sb[:, c, HALF:T], xt[:, c, HALF:T])
            prefetch_w(1)
            wv_sb = constp.tile([P, CK, 2, 4 * P], BF16)
            for c in range(CK):
                nc.sync.dma_start(wv_sb[:, c, :, :], wv[:, c, :, :])
            prefetch_w(2)
            # [s_p, head, s_tile, 64 v cols + 1 ones col]
            v_all = constp.tile([P, H, TT, HS + 1], BF16)
            nc.gpsimd.memset(v_all[:, :, :, HS:HS + 1], 1.0)

            qk = {}

            def emit_qk(pair, g_outer=False):
                wq_sb, wk_sb = wtiles.pop(pair)
                qT = qkp.tile([P, T], BF16, tag="qk", name=f"q{pair}")
                kT = qkp.tile([P, T], BF16, tag="qk", name=f"k{pair}")
                # g_outer (pair 0): do both projections' g=0 halves first so
                # the start-of-kernel matmuls only need the first xt halves
                order = (((wq_sb, qT, 0), (wk_sb, kT, 0),
                          (wq_sb, qT, 1), (wk_sb, kT, 1)) if g_outer else
                         ((wq_sb, qT, 0), (wq_sb, qT, 1),
                          (wk_sb, kT, 0), (wk_sb, kT, 1)))
                for wsb, dst, g in order:
                    pp = psA.tile([P, HALF], F32, tag="ps",
                                  name=f"pp{pair}_{id(dst)}_{g}")
                    for c in range(CK):
                        nc.tensor.matmul(
                            pp[:, :],
                            wsb[:, c, :],
                            xt_sb[:, c, ds(HALF * g, HALF)],
                            start=(c == 0),
                            stop=(c == CK - 1),
                        )
                    nc.vector.tensor_copy(dst[:, ds(HALF * g, HALF)], pp[:, :])
                qk[pair] = (qT, kT)

            def emit_v_group(g, jr):
                # v for heads 8g..8g+7 (pairs 4g..4g+3), t tiles in jr
                for j in jr:
                    pv = psA.tile([P, HALF], F32, tag="ps", name=f"pv{g}_{j}")
                    for c in range(CK):
                        nc.tensor.matmul(
                            pv[:, :],
                            xt_sb[:, c, ts(j, P)],
                            wv_sb[:, c, g, :],
                            start=(c == 0),
                            stop=(c == CK - 1),
                        )
                    # pv cols are (8 heads of group) x 64 in order
                    nc.vector.tensor_copy(
                        v_all[:, ds(8 * g, 8), j, 0:HS],
                        pv.rearrange("p (h d) -> p h d", d=HS),
                    )

            es_tiles = {}

            def emit_scores_exp(pair, ir=range(TT), part="ab"):
                if pair not in es_tiles:
                    qT, kT = qk[pair]
                    es = [expp.tile([P, 2, T], BF16, tag="exp",
                                    name=f"e{pair}_{i}") for i in range(TT)]
                    es_tiles[pair] = (qT, kT, es)
                qT, kT, es = es_tiles[pair]
                for i in ir:
                    t0 = P * i
                    spans = [(t0, HALF), (HALF, T)] if t0 < HALF else [(t0, T)]
                    if t0 < HALF and part == "a":
                        spans = spans[:1]
                    elif t0 < HALF and part == "b":
                        spans = spans[1:]
                    for a, b in spans:
                        sc = psSc.tile([P, 2, HALF], F32, tag="sc",
                                       name=f"sc{pair}_{i}_{a}")
                        for w in range(2):
                            nc.tensor.matmul(
                                sc[:, w, 0:b - a],
                                kT[ds(HS * w, HS), ds(t0, P)],
                                qT[ds(HS * w, HS), ds(a, b - a)],
                            )
                        # one exp for BOTH heads (scale = 1/sqrt(hs))
                        nc.scalar.activation(
                            es[i][:, :, a:b],
                            sc[:, :, 0:b - a],
                            mybir.ActivationFunctionType.Exp,
                            scale=HS ** -0.5,
                        )
                        if a == t0:
                            # in-place causal mask of the diagonal block:
                            # keep where t - s >= 0, else 0
                            nc.gpsimd.affine_select(
                                out=es[i][:, :, t0:t0 + P],
                                in_=es[i][:, :, t0:t0 + P],
                                pattern=[[0, 2], [1, P]],
                                compare_op=mybir.AluOpType.is_ge,
                                fill=0.0,
                                base=0,
                                channel_multiplier=-1,
                            )

            def emit_av(pair, hhs=(0, 1), split_copies=False,
                        copy_scalar=False):
                _, _, es = es_tiles[pair]
                for hh in hhs:
                    avp = [psV.tile([HS + 1, HALF], F32, tag="av",
                                    name=f"av{pair}_{hh}_{w}") for w in range(2)]
                    contrib = [i for i in range(TT) if P * i < HALF * (hh + 1)]
                    for idx, i in enumerate(contrib):
                        g0 = max(HALF * hh, P * i)
                        g1 = HALF * (hh + 1)
                        for w in range(2):
                            nc.tensor.matmul(
                                avp[w][:, ds(g0 - HALF * hh, g1 - g0)],
                                v_all[:, 2 * pair + w, i, :],
                                es[i][:, w, ds(g0, g1 - g0)],
                                start=(idx == 0),
                                stop=(idx == len(contrib) - 1),
                            )
                    for w in range(2):
                        h = 2 * pair + w
                        avs = smallp.tile([HS + 1, HALF], BF16, tag="avs",
                                          name=f"avs{pair}_{hh}_{w}")
                        if copy_scalar:
                            # last pair: ScalarE is idle once exp(7) is done
                            # and its PSUM->SBUF copy is cheaper than DVE's;
                            # runs parallel to DVE finishing earlier pairs
                            nc.scalar.copy(avs[:, :], avp[w][:, :])
                            nc.sync.dma_start(out[h, :, ds(HALF * hh, HALF)],
                                              avs[:, :])
                        elif split_copies and hh == 1:
                            # half-granular copy->DMA so the last out DMA
                            # starts before the full PSUM drain finishes
                            for q in range(2):
                                nc.vector.tensor_copy(
                                    avs[:, ds(256 * q, 256)],
                                    avp[w][:, ds(256 * q, 256)])
                                nc.sync.dma_start(
                                    out[h, :, ds(HALF * hh + 256 * q, 256)],
                                    avs[:, ds(256 * q, 256)])
                        else:
                            nc.vector.tensor_copy(avs[:, :], avp[w][:, :])
                            nc.sync.dma_start(out[h, :, ds(HALF * hh, HALF)],
                                              avs[:, :])

            # schedule: AV(p) emitted one pair late so PE never waits on
            # ScalarE exp; V-proj groups split in j-halves spread at pairs
            # 0/1/3/5 (vg0 complete before AV(0) at pair 1; vg1 before
            # AV(4) at pair 5).
            V_AT = {0: (0, range(0, 4)), 1: (0, range(4, 8)),
                    3: (1, range(0, 4)), 5: (1, range(4, 8))}
            emit_qk(0)
            for pair in range(PAIRS - 1):
                emit_scores_exp(pair)
                prefetch_w(pair + 3)
                emit_qk(pair + 1)
                if pair in V_AT:
                    emit_v_group(*V_AT[pair])
                if pair >= 1:
                    emit_av(pair - 1)
            # pair 7 tail: stagger the last pair's score spans between
            # AV(6)'s halves so exp(7) overlaps AV work instead of
            # trailing it
            emit_scores_exp(7, range(0, 4))
            emit_av(6, (0,))
            emit_scores_exp(7, range(4, 8))
            emit_av(6, (1,))
            emit_av(7, copy_scalar=True)
    nc.compile()
    return nc


def get_nc():
    global _BUILT
    if _BUILT is None:
        _BUILT = build_nc()
    return _BUILT


def prep_inputs(x, Wq, Wk, Wv):
    """Host-side shard + layout prep. Returns in_maps (one dict per core)."""
    x = np.asarray(x, dtype=np.float32)
    Wq = np.asarray(Wq, dtype=np.float32)
    Wk = np.asarray(Wk, dtype=np.float32)
    Wv = np.asarray(Wv, dtype=np.float32)
    bf = ml_dtypes.bfloat16

    # xT[b]: [C, T] -> [p, c, t] with row 128c+p
    xts = []
    for b in range(B):
        xT = np.ascontiguousarray(x[b].T)          # [C, T]
        xts.append(xT.reshape(CK, P, T).transpose(1, 0, 2).astype(bf))

    def pack_pairs(W):
        # [H, C, hs] -> [pair, C, 128] -> [pair, p, c, f]
        Wp = W.reshape(PAIRS, 2, C, HS).transpose(0, 2, 1, 3).reshape(PAIRS, C, P)
        return Wp.reshape(PAIRS, CK, P, P).transpose(0, 2, 1, 3)  # [pair, p, c, f]

    wq_p = pack_pairs(Wq)
    wk_p = pack_pairs(Wk)
    wqk_host = np.stack([wq_p, wk_p], axis=0).astype(bf)  # [2, pair, p, c, f]
    # wv: [p, c, group(2), 512] with cols = 8 heads x 64
    wv_g = Wv.reshape(2, 8, C, HS).transpose(2, 0, 1, 3).reshape(C, 2, 4 * P)
    wv_host = np.ascontiguousarray(
        wv_g.reshape(CK, P, 2, 4 * P).transpose(1, 0, 2, 3)).astype(bf)

    return [
        {"xt": np.ascontiguousarray(xts[b]), "wqk": wqk_host, "wv": wv_host}
        for b in range(B)
    ]


def run_on_device(in_maps, **kwargs):
    nc = get_nc()
    return run_bass_kernel_spmd(nc, in_maps, list(range(B)), **kwargs)


def assemble(core_out):
    """[H, 65, T] raw out^T -> normalize on host -> [T, H*HS]."""
    o = np.asarray(core_out, dtype=np.float32)
    num = o[:, :HS, :]                       # [H, 64, T]
    den = o[:, HS:HS + 1, :]                 # [H, 1, T]
    y = num / den
    return np.ascontiguousarray(y.transpose(2, 0, 1).reshape(T, H * HS))


def kernel(x, Wq, Wk, Wv):
    in_maps = prep_inputs(x, Wq, Wk, Wv)
    res = run_on_device(in_maps)
    return np.stack([assemble(res.results[b]["out"]) for b in range(B)], axis=0)
